# revision 7
# baseline (speedup 1.0000x reference)
"""GPT-NeoX attention layer (B=2, S=2048, E=2048, H=16, partial RoPE 32/128)
as a Bass/Tile kernel for 8 Trainium2 NeuronCores.

Sharding: tensor-parallel across heads (2 heads per core, Megatron-style).
Each core computes QKV projection for its 768 rows of w_qkv, applies partial
RoPE, runs causal attention for its 2 heads x 2 batches, and produces a
partial dense output (contraction over its 256 columns of w_dense).  The 8
partial outputs are summed on the host (no on-device collectives needed) and
the dense bias is added once on the host.

All device matmuls keep fp32 data in SBUF; the tensor engine runs them as
float32r (1 cycle/row for N>=256) with fp32 PSUM accumulation.

Layout choices (everything transposed once on the host so the contraction dim
always lands on SBUF partitions; no on-device transposes of x or weights):
  xT      [E, B*S]    x flattened and transposed
  wqkvT   [E, 768]    per-core slice of w_qkv, transposed
  wdT     [256, E]    per-core column-slice of w_dense, transposed
  qkvT    [768, B*S]  phase-1 output: per-head Q^T,K^T,V^T row blocks
  scores  S^T = (K^T)^T @ (Q^T) in [sk, sq] layout; softmax sums over the
          partition dim via a ones-matmul; y^T accumulated directly as
          V_nat^T @ P^T, which is the layout the dense matmul consumes.
"""

import numpy as np
from contextlib import ExitStack

import concourse.bass as bass
import concourse.bacc as bacc
import concourse.mybir as mybir
import concourse.tile as tile
from concourse.masks import make_identity

AF = mybir.ActivationFunctionType
F32 = mybir.dt.float32
F32R = mybir.dt.float32r

NEG_MASK = -1.0e9


class Cfg:
    def __init__(self, B=2, S=2048, E=2048, H=16, n_cores=8, mm_dtype=F32R):
        self.B, self.S, self.E, self.H = B, S, E, H
        self.HS = 128                 # head size (fixed: one partition tile)
        self.ROT = 32                 # rotary dims
        self.n_cores = n_cores
        self.HPC = H // n_cores       # heads per core
        self.R = 3 * self.HS * self.HPC   # per-core qkv rows
        self.SF = B * S               # flattened sequence
        self.KT = E // 128            # contraction tiles for qkv proj
        self.RT = self.R // 128       # row tiles of per-core qkv
        self.CW = self.HPC * self.HS  # per-core dense contraction width
        self.CT = self.CW // 128
        self.EO = E // 128            # dense output row tiles
        self.SCALE = 1.0 / np.sqrt(self.HS)
        self.mm_dtype = mm_dtype
        assert self.SF % 4 == 0 and S % 512 == 0 and E % 128 == 0


def build_program(cfg: Cfg) -> bass.Bass:
    B, S, E = cfg.B, cfg.S, cfg.E
    SF, R, KT, RT = cfg.SF, cfg.R, cfg.KT, cfg.RT
    HPC, ROT = cfg.HPC, cfg.ROT
    G = SF // 4                      # rope partition-regroup chunk
    mmdt = cfg.mm_dtype

    def rc(ap):
        # walrus requires every producer of an FP32r matmul operand to emit
        # FP32r (round-to-tf32); bitcast keeps the underlying tile fp32
        return ap.bitcast(mmdt) if mmdt == F32R else ap

    nc = bacc.Bacc(None)
    xT = nc.dram_tensor("xT", [E, SF], F32, kind="ExternalInput")
    wqkvT = nc.dram_tensor("wqkvT", [E, R], F32, kind="ExternalInput")
    bqkv = nc.dram_tensor("bqkv", [R], F32, kind="ExternalInput")
    wdT = nc.dram_tensor("wdT", [cfg.CW, E], F32, kind="ExternalInput")
    cos128 = nc.dram_tensor("cos128", [128, G], F32, kind="ExternalInput")
    sin128s = nc.dram_tensor("sin128s", [128, G], F32, kind="ExternalInput")
    maskT = nc.dram_tensor("maskT", [128, 128], F32, kind="ExternalInput")
    outT = nc.dram_tensor("outT", [E, SF], F32, kind="ExternalOutput")

    with tile.TileContext(nc) as tc, ExitStack() as stk:
        consts = stk.enter_context(tc.tile_pool(name="consts", bufs=1))
        qkvp = stk.enter_context(tc.tile_pool(name="qkvbuf", bufs=1))
        qkv_sb = qkvp.tile([128, RT, SF], F32)

        ident = consts.tile([128, 128], F32)
        make_identity(nc, ident)
        ones_k = consts.tile([128, 1], F32)    # lhsT for partition sums
        ones_m = consts.tile([1, 128], F32)    # lhsT for partition broadcast
        ones_tmp = consts.tile([128, 128], F32, tag="onestmp")
        nc.vector.memset(ones_tmp, 1.0)
        nc.vector.tensor_copy(rc(ones_k[:, :]), ones_tmp[:, 0:1])
        nc.vector.tensor_copy(rc(ones_m[:, :]), ones_tmp[0:1, :])
        mask_sb = consts.tile([128, 128], F32)
        nc.sync.dma_start(out=mask_sb, in_=maskT[:, :])
        bq_sb = consts.tile([128, RT], F32)
        nc.sync.dma_start(out=bq_sb, in_=bqkv.rearrange("(rt p) -> p rt", p=128))

        # ---------------- Phase 1: QKV projection -> qkvT in SBUF ----------
        SC = 256
        with tc.tile_pool(name="wq", bufs=1) as wp, \
             tc.tile_pool(name="xs", bufs=2) as xp, \
             tc.tile_pool(name="ps1", bufs=4, space="PSUM") as pp1:
            w_sb = wp.tile([128, KT, R], F32)
            nc.sync.dma_start(
                out=rc(w_sb[:, :, :]),
                in_=rc(wqkvT.rearrange("(kt p) r -> p kt r", p=128)))
            x_view = xT.rearrange("(kt p) s -> p kt s", p=128)
            for sc in range(SF // SC):
                xt = xp.tile([128, KT, SC], F32)
                nc.sync.dma_start(out=rc(xt[:, :, :]),
                                  in_=rc(x_view[:, :, sc * SC:(sc + 1) * SC]))
                for rt in range(RT):
                    ps = pp1.tile([128, SC], F32)
                    for kt in range(KT):
                        nc.tensor.matmul(
                            ps,
                            w_sb[:, kt, rt * 128:(rt + 1) * 128].bitcast(mmdt),
                            xt[:, kt, :].bitcast(mmdt),
                            start=(kt == 0), stop=(kt == KT - 1))
                    # evict + qkv bias (per-partition) on ScalarE
                    nc.scalar.activation(
                        rc(qkv_sb[:, rt, sc * SC:(sc + 1) * SC]), ps,
                        AF.Identity, bias=bq_sb[:, rt:rt + 1])

        # ---------------- RoPE on first ROT rows of each Q^T / K^T ---------
        # Regroup [ROT, SF] -> [128, SF/4] so the DVE ops use all lanes.
        with tc.tile_pool(name="rope", bufs=2) as rp:
            cos_sb = rp.tile([128, G], F32, tag="costab")
            sin_sb = rp.tile([128, G], F32, tag="sintab")
            nc.sync.dma_start(out=cos_sb, in_=cos128[:, :])
            nc.sync.dma_start(out=sin_sb, in_=sin128s[:, :])
            half = ROT // 2
            for h in range(HPC):
                for qk in range(2):
                    rt = 3 * h + qk
                    blk = qkv_sb[0:ROT, rt, :]
                    plain = rp.tile([128, G], F32, tag="plain")
                    sw = rp.tile([128, G], F32, tag="swap")
                    for g in range(4):
                        gs = slice(g * G, (g + 1) * G)
                        nc.sync.dma_start(
                            out=plain[g * 32:(g + 1) * 32, :], in_=blk[:, gs])
                        # rotate_half: rows 0:16 <- rows 16:32, rows 16:32 <- 0:16
                        nc.sync.dma_start(
                            out=sw[g * 32:g * 32 + half, :],
                            in_=qkv_sb[half:ROT, rt, gs])
                        nc.sync.dma_start(
                            out=sw[g * 32 + half:(g + 1) * 32, :],
                            in_=qkv_sb[0:half, rt, gs])
                    nc.vector.tensor_mul(plain, plain, cos_sb)
                    nc.vector.tensor_mul(sw, sw, sin_sb)   # sign folded in table
                    nc.vector.tensor_add(rc(plain[:, :]), plain, sw)
                    for g in range(4):
                        gs = slice(g * G, (g + 1) * G)
                        nc.sync.dma_start(
                            out=rc(qkv_sb[0:ROT, rt, gs]),
                            in_=rc(plain[g * 32:(g + 1) * 32, :]))

        # ---------------- Phase 2+3: attention + partial dense -------------
        NCH = S // 512                    # sq chunks per (b, h) pair
        with tc.tile_pool(name="yt", bufs=1) as yp, \
             tc.tile_pool(name="wd", bufs=1) as wdp, \
             tc.tile_pool(name="vnat", bufs=2) as vp, \
             tc.tile_pool(name="pstrip", bufs=3) as ppool, \
             tc.tile_pool(name="norm", bufs=2) as npool, \
             tc.tile_pool(name="outsb", bufs=4) as op, \
             tc.tile_pool(name="psA", bufs=2, space="PSUM") as psA, \
             tc.tile_pool(name="psY", bufs=2, space="PSUM") as psY, \
             tc.tile_pool(name="psS", bufs=2, space="PSUM") as psS, \
             tc.tile_pool(name="psD", bufs=2, space="PSUM") as psD:
            yT_sb = yp.tile([128, HPC, SF], F32)
            wd_sb = wdp.tile([128, cfg.CT, E], F32)
            nc.sync.dma_start(
                out=rc(wd_sb[:, :, :]),
                in_=rc(wdT.rearrange("(ct p) e -> p ct e", p=128)))

            def dense_cols(b):
                # partial dense for columns of batch b, overlaps next pair
                sc0 = b * (S // 512)
                for eo in range(cfg.EO):
                    for scn in range(S // 512):
                        col = b * S + scn * 512
                        ps = psD.tile([128, 512], F32)
                        for ct in range(cfg.CT):
                            nc.tensor.matmul(
                                ps,
                                wd_sb[:, ct, eo * 128:(eo + 1) * 128].bitcast(mmdt),
                                yT_sb[:, ct, col:col + 512].bitcast(mmdt),
                                start=(ct == 0), stop=(ct == cfg.CT - 1))
                        ot = op.tile([128, 512], F32)
                        if (eo + scn) % 2 == 0:
                            nc.vector.tensor_copy(ot, ps)
                        else:
                            nc.scalar.activation(ot, ps, AF.Copy)
                        nc.sync.dma_start(
                            out=outT[eo * 128:(eo + 1) * 128, col:col + 512],
                            in_=ot)

            for b in range(B):
                for h in range(HPC):
                    scol = b * S
                    q_t = qkv_sb[:, 3 * h + 0, scol:scol + S]
                    k_t = qkv_sb[:, 3 * h + 1, scol:scol + S]
                    v_t = qkv_sb[:, 3 * h + 2, scol:scol + S]
                    njt = S // 128
                    # V natural layout [sk, d] via PE transpose of V^T tiles
                    vnat = vp.tile([128, njt, 128], F32)
                    for jt in range(njt):
                        pst = psA.tile([128, 512], F32, tag="A")
                        nc.tensor.transpose(
                            pst[:, 0:128], v_t[:, jt * 128:(jt + 1) * 128], ident)
                        nc.vector.tensor_copy(rc(vnat[:, jt, :]), pst[:, 0:128])
                    for c in range(NCH):
                        yacc = psY.tile([128, 512], F32)
                        sums = psS.tile([1, 512], F32)
                        nj = 4 * c + 4
                        for j in range(nj):
                            off = max(0, j * 128 - c * 512)
                            n = 512 - off
                            first, last = (j == 0), (j == nj - 1)
                            ps = psA.tile([128, 512], F32, tag="A")
                            nc.tensor.matmul(
                                ps[:, off:],
                                k_t[:, j * 128:(j + 1) * 128].bitcast(mmdt),
                                q_t[:, c * 512 + off:c * 512 + 512].bitcast(mmdt),
                                start=True, stop=True, skip_group_check=True)
                            if j >= 4 * c:  # diagonal block: causal mask
                                nc.vector.tensor_add(
                                    ps[:, off:off + 128], ps[:, off:off + 128],
                                    mask_sb)
                            pT = ppool.tile([128, 512], F32)
                            nc.scalar.activation(
                                rc(pT[:, off:]), ps[:, off:], AF.Exp,
                                scale=cfg.SCALE)
                            nc.tensor.matmul(
                                sums[:, off:], ones_k.bitcast(mmdt),
                                pT[:, off:].bitcast(mmdt),
                                start=first, stop=last, skip_group_check=True)
                            nc.tensor.matmul(
                                yacc[:, off:], vnat[:, j, :].bitcast(mmdt),
                                pT[:, off:].bitcast(mmdt),
                                start=first, stop=last, skip_group_check=True)
                        # normalize: recip of sums, broadcast over partitions
                        recip = npool.tile([1, 512], F32, tag="recip")
                        with nc.allow_low_precision(
                                reason="tf32 rounding for fp32r matmul"):
                            nc.vector.reciprocal(rc(recip[:, :]), sums)
                        bc = psA.tile([128, 512], F32, tag="A")
                        nc.tensor.matmul(
                            bc, ones_m.bitcast(mmdt), recip.bitcast(mmdt),
                            start=True, stop=True, skip_group_check=True)
                        bcs = npool.tile([128, 512], F32, tag="bcs")
                        nc.vector.tensor_copy(bcs, bc)
                        nc.vector.tensor_mul(
                            rc(yT_sb[:, h, scol + c * 512:scol + (c + 1) * 512]),
                            yacc, bcs)
                dense_cols(b)

    nc.finalize()
    return nc


# ---------------------------------------------------------------------------
# Host-side input preparation / sharding
# ---------------------------------------------------------------------------

def _tf32_round(a: np.ndarray) -> np.ndarray:
    """Round fp32 to tf32 (round-to-nearest-even on the low 13 mantissa bits).
    DMA cannot round, so FP32r matmul operands fed straight from DRAM are
    pre-rounded on the host."""
    u = np.ascontiguousarray(a, np.float32).view(np.uint32)
    u = (u + 0x0FFF + ((u >> 13) & 1)) & np.uint32(0xFFFFE000)
    return u.view(np.float32)


def _rope_tables(cfg: Cfg):
    inv_freq = 1.0 / (10000.0 ** (np.arange(0, cfg.ROT, 2, dtype=np.float64)
                                  / cfg.ROT))
    t = np.arange(cfg.S, dtype=np.float64)
    freqs = np.outer(t, inv_freq)                       # [S, 16]
    emb = np.concatenate([freqs, freqs], axis=-1)       # [S, 32]
    cos = np.cos(emb).T.astype(np.float32)              # [32, S]
    sin = np.sin(emb).T.astype(np.float32)
    cosF = np.tile(cos, (1, cfg.B))                     # [32, SF]
    sinF = np.tile(sin, (1, cfg.B))
    sinF[:cfg.ROT // 2] *= -1.0                         # fold rotate_half sign
    G = cfg.SF // 4
    cos128 = np.ascontiguousarray(
        cosF.reshape(32, 4, G).transpose(1, 0, 2).reshape(128, G))
    sin128s = np.ascontiguousarray(
        sinF.reshape(32, 4, G).transpose(1, 0, 2).reshape(128, G))
    return cos128, sin128s


def make_in_maps(cfg: Cfg, x, w_qkv, b_qkv, w_dense):
    rnd = _tf32_round if cfg.mm_dtype == F32R else (
        lambda a: np.ascontiguousarray(a, np.float32))
    xT = rnd(x.reshape(cfg.B * cfg.S, cfg.E).T)
    cos128, sin128s = _rope_tables(cfg)
    p = np.arange(128)[:, None]
    f = np.arange(128)[None, :]
    maskT = np.where(p <= f, 0.0, NEG_MASK).astype(np.float32)
    in_maps = []
    for i in range(cfg.n_cores):
        rows = slice(i * cfg.R, (i + 1) * cfg.R)
        cols = slice(i * cfg.CW, (i + 1) * cfg.CW)
        in_maps.append({
            "xT": xT,
            "wqkvT": rnd(w_qkv[rows, :].T),
            "bqkv": np.ascontiguousarray(b_qkv[rows]).astype(np.float32),
            "wdT": rnd(w_dense[:, cols].T),
            "cos128": cos128,
            "sin128s": sin128s,
            "maskT": maskT,
        })
    return in_maps


def combine_outputs(cfg: Cfg, results, b_dense):
    acc = np.zeros((cfg.E, cfg.SF), dtype=np.float64)
    for r in results:
        acc += r["outT"].astype(np.float64)
    out = acc.T.reshape(cfg.B, cfg.S, cfg.E) + b_dense.astype(np.float64)
    return out.astype(np.float32)


_PROGRAM_CACHE = {}


def kernel(x, w_qkv, b_qkv, w_dense, b_dense):
    from concourse.bass_utils import run_bass_kernel_spmd

    cfg = Cfg()
    key = "full"
    if key not in _PROGRAM_CACHE:
        _PROGRAM_CACHE[key] = build_program(cfg)
    nc = _PROGRAM_CACHE[key]
    in_maps = make_in_maps(cfg, np.asarray(x), np.asarray(w_qkv),
                           np.asarray(b_qkv), np.asarray(w_dense))
    res = run_bass_kernel_spmd(nc, in_maps, list(range(cfg.n_cores)))
    return combine_outputs(cfg, res.results, np.asarray(b_dense))
